# revision 6
# baseline (speedup 1.0000x reference)
"""Trainium2 Bass kernel for LoopyBeliefPropagation (3-iter, mask=ones).

Same math as the baseline kernel but with the tail algebra collapsed into
a linear form so almost nothing runs after the streamed softplus:

  sp[d,h,s] = softplus(t[d,h,s]) - ln2      (Exp pass; Ln(0.5*y+0.5) pass)
  RS[d,h] = sum_s sp, CS[p,h] = sum_d sp[d,h,p]
  bd = A6 + CS*C1 + (CS-RS)*C2 - M(CS*OME)
  out = [sigma(-bd), sigma(bd)]

with C1 = 15878+252E, C2 = 127+E host constants, M(v) = ones-matmul
column-broadcast, and A6 a [128,64] tensor precomputed on device from
s_edge and the gathered diagonal/h-column/row-h side values.  The
gathered values ride the main stream as two extra leading h-slots, so
their softplus needs no extra ACT instructions and is ready right after
the first Ln chunk.  Streaming pipelines Exp/Ln against DMA; row-sum
reduces run on DVE, the per-chunk tail algebra on Pool (gpsimd), column
sums and broadcasts on PE.

Sharding: 8 cores x (b in 0..3, h-half in {0:64, 64:128}).
"""

import numpy as np

L = 128
H = 64            # h-slices per core
GX = 2            # leading ext slots carrying the gathered side values
EXT = H + GX
N_CORES = 8

# streaming configuration (ext-slot units; slot = one [128,128] plane)
DMA_CH = [2, 4, 4, 8, 8, 8, 8, 8, 8, 4, 2, 2]
EXP_CH = [2, 4, 4, 8, 8, 8, 8, 8, 8, 4, 2, 2]
LN_CH = [2, 8, 16, 16, 8, 8, 4, 4]
SIG_AT = [5, 7]      # sigma/odma group ends (ln chunk indices)
LN_DELAY = 0         # emit ln chunks this many exp chunks late
SMALLS_AFTER_DMA = 4   # emit se/aux DMAs after this t chunk

# aux column layout: E | NF | OME | C1 | C2 | C12 | IDENT
A_E, A_N, A_OME, A_C1, A_C2, A_C12, A_ID = 0, 64, 128, 192, 256, 320, 384
A_COLS = 512

_PROGRAM = None


def _build_program():
    import concourse.bacc as bacc
    import concourse.mybir as mybir
    import concourse.tile as tile

    fp32 = mybir.dt.float32
    AF = mybir.ActivationFunctionType
    OP = mybir.AluOpType

    # Exp and Ln live in one PWP table; without this filter the table
    # chooser reloads the ACT table between every Exp/Ln pair.
    if not getattr(bacc, "_lbp_act_tables_patched", False):
        _orig_tables = bacc.get_activation_tables

        def _ln_exp_only(arch):
            t = _orig_tables(arch)
            exp_ln = {AF.Exp, AF.Ln}
            return {
                name: (funcs if name == "natural_log_exp_and_others"
                       else set(funcs) - exp_ln)
                for name, funcs in t.items()
            }

        bacc.get_activation_tables = _ln_exp_only
        bacc._lbp_act_tables_patched = True

    nc = bacc.Bacc(None, target_bir_lowering=False)

    t_d = nc.dram_tensor("t", [L, EXT, L], fp32, kind="ExternalInput")
    se_d = nc.dram_tensor("se", [L, H, 2], fp32, kind="ExternalInput")
    aux_d = nc.dram_tensor("aux", [L, A_COLS], fp32, kind="ExternalInput")
    o_d = nc.dram_tensor("o", [L, H, 2], fp32, kind="ExternalOutput")

    exp_cum = np.cumsum([0] + EXP_CH).tolist()
    ln_cum = np.cumsum([0] + LN_CH).tolist()
    dma_cum = np.cumsum([0] + DMA_CH).tolist()
    assert exp_cum[-1] == EXT and ln_cum[-1] == EXT and dma_cum[-1] == EXT
    assert all(c in exp_cum for c in ln_cum)
    assert all(c in dma_cum for c in exp_cum)
    assert ln_cum[1] >= GX

    with tile.TileContext(nc) as tc:
        with (
            tc.tile_pool(name="sb", bufs=1) as sb,
            tc.tile_pool(name="psum", bufs=1, space="PSUM") as pp,
        ):
            tfull = sb.tile([L, EXT, L], fp32, tag="tfull")
            spf = sb.tile([L, EXT, L], fp32, tag="spf")
            aux = sb.tile([L, A_COLS], fp32, tag="aux")
            se = sb.tile([L, H, 2], fp32, tag="se")
            ones = sb.tile([L, L], fp32, tag="ones")
            half = sb.tile([L, 1], fp32, tag="half")
            dmy = sb.tile([L, 1], fp32, tag="dmy")

            def w64(tag):
                return sb.tile([L, H], fp32, tag=tag, name=tag)

            PD, A1, u0, u1, A3, A5 = (w64(t) for t in
                                      ["PD", "A1", "u0", "u1", "A3", "A5"])
            K1, K2, K3, A6, tp0, tp1 = (w64(t) for t in
                                        ["K1", "K2", "K3", "A6", "tp0", "tp1"])
            RS, ZZ, X, U, V, W = (w64(t) for t in
                                  ["RS", "ZZ", "X", "U", "V", "W"])
            CSs = w64("CSs")
            scrA = sb.tile([L, 16, 64], fp32, tag="scrA")
            scrB = sb.tile([L, 16, 32], fp32, tag="scrB")
            bd, bdc, e1, ssum = (w64(t) for t in ["bd", "bdc", "e1", "ssum"])
            osb = sb.tile([L, H, 2], fp32, tag="osb")

            cs_ps = pp.tile([L, H], fp32, tag="cs_ps")
            s0_ps = pp.tile([L, H], fp32, tag="s0_ps")
            s1_ps = pp.tile([L, H], fp32, tag="s1_ps")
            sk_ps = pp.tile([L, H], fp32, tag="sk_ps")
            bm_ps = pp.tile([L, H], fp32, tag="bm_ps")

            E = aux[:, A_E:A_E + H]
            NF = aux[:, A_N:A_N + H]
            OME = aux[:, A_OME:A_OME + H]
            C1 = aux[:, A_C1:A_C1 + H]
            C2 = aux[:, A_C2:A_C2 + H]
            C12 = aux[:, A_C12:A_C12 + H]
            IDT = aux[:, A_ID:A_ID + L]
            # gathered softplus values ride ext slots 0-1 of the stream
            G = spf[:, 0, 0:H]
            DG = spf[:, 0, H:2 * H]
            ROWH = spf[:, 1, 0:H]

            nc.sync.dma_start(tfull[:, 0:dma_cum[1], :],
                              t_d[:, 0:dma_cum[1], :])
            nc.gpsimd.memset(ones[:], 1.0)
            nc.gpsimd.memset(half[:], 0.5)

            # dummy act to trigger the Exp/Ln table load at t=0
            nc.scalar.activation(dmy[:], ones[:, 0:1], AF.Exp, scale=0.0)

            # DMAs: t chunks stream; small inputs slotted mid-stream
            for k in range(1, len(DMA_CH)):
                h0, h1 = dma_cum[k], dma_cum[k + 1]
                nc.sync.dma_start(tfull[:, h0:h1, :], t_d[:, h0:h1, :])
                if k == SMALLS_AFTER_DMA:
                    nc.sync.dma_start(se[:], se_d[:])
                    nc.sync.dma_start(aux[:], aux_d[:])

            # ---- ACT stream + per-chunk reductions/tails ----
            li = 0

            def emit_exp(k):
                h0, h1 = exp_cum[k], exp_cum[k + 1]
                nc.scalar.activation(spf[:, h0:h1, :], tfull[:, h0:h1, :],
                                     AF.Exp)

            def emit_pre1():
                # s_edge part of the A6 chain (DVE); needs only se + aux
                nc.vector.tensor_sub(PD[:], se[:, :, 1], se[:, :, 0])
                nc.vector.tensor_mul(A1[:], PD[:], C2)
                nc.vector.tensor_mul(u0[:], PD[:], OME)
                nc.tensor.matmul(s0_ps[:], ones[:], u0[:],
                                 start=True, stop=True)
                nc.vector.tensor_mul(u1[:], A1[:], OME)
                nc.tensor.matmul(s1_ps[:], ones[:], u1[:],
                                 start=True, stop=True)
                nc.vector.tensor_mul(A3[:], A1[:], NF)
                nc.vector.scalar_tensor_tensor(
                    A3[:], PD[:], 2.0, A3[:], op0=OP.mult, op1=OP.add)
                nc.vector.tensor_mul(tp0[:], E, PD[:])
                nc.vector.tensor_sub(A3[:], A3[:], tp0[:])
                nc.vector.tensor_sub(A3[:], A3[:], s0_ps[:])
                nc.vector.tensor_add(A5[:], A3[:], PD[:])
                nc.vector.tensor_mul(A5[:], A5[:], NF)
                nc.vector.tensor_add(A5[:], A5[:], u1[:])
                nc.vector.tensor_add(A5[:], A5[:], PD[:])
                nc.vector.tensor_sub(A5[:], A5[:], s1_ps[:])

            def emit_pre2():
                # gathered-values part of A6 (DVE); needs ext slots 0-1
                nc.vector.tensor_add(K1[:], G, DG)
                nc.vector.tensor_mul(tp1[:], E, G)
                nc.vector.tensor_sub(K1[:], K1[:], tp1[:])
                nc.vector.tensor_add(K2[:], ROWH, DG)
                nc.vector.tensor_mul(tp1[:], E, DG)
                nc.vector.tensor_sub(K2[:], K2[:], tp1[:])
                nc.vector.tensor_sub(K3[:], K1[:], K2[:])
                nc.vector.tensor_mul(tp1[:], K2[:], C1)
                nc.vector.tensor_sub(A6[:], tp1[:], A5[:])
                nc.vector.tensor_mul(tp1[:], K3[:], C2)
                nc.vector.tensor_sub(A6[:], A6[:], tp1[:])
                nc.vector.tensor_mul(tp0[:], K2[:], OME)
                # sk collects M(K2*OME) + M(G): the M(G) term converts the
                # tail's M(CS*OME) into a plain M(RS), removing the
                # per-chunk CS*OME op.
                nc.tensor.matmul(sk_ps[:], ones[:], tp0[:],
                                 start=True, stop=False)
                nc.tensor.matmul(sk_ps[:], ones[:], G,
                                 start=False, stop=True)
                nc.vector.tensor_sub(A6[:], A6[:], sk_ps[:])

            def emit_ln(k):
                h0, h1 = ln_cum[k], ln_cum[k + 1]
                # sp = Ln(0.5*e^t + 0.5) = softplus(t) - ln2
                nc.scalar.activation(spf[:, h0:h1, :], spf[:, h0:h1, :],
                                     AF.Ln, bias=half[:], scale=half[:])

            def emit_chunk_post(k):
                # bd = A6' + CS*C12 - RS*C2 - M(RS), with A6' carrying the
                # M(G) and M(K2*OME) broadcast terms.
                # PE: colsums + M(RS); DVE: U/RS/bd/clamp; Pool: UA/V/W.
                x0, x1 = max(ln_cum[k], GX), ln_cum[k + 1]
                if x1 <= x0:
                    return
                h0, h1 = x0 - GX, x1 - GX
                sl = (slice(None), slice(h0, h1))
                for j in range(x0, x1):
                    nc.tensor.matmul(cs_ps[:, j - GX:j - GX + 1],
                                     spf[:, j, :], ones[:, 0:1],
                                     start=True, stop=True)
                # bm psum: preload -A6 (identity stationary), add M(RS)
                nc.tensor.matmul(bm_ps[sl], IDT, A6[sl],
                                 start=True, stop=False)
                nc.vector.tensor_reduce(RS[sl], spf[:, x0:x1, :],
                                        axis=mybir.AxisListType.X, op=OP.add)
                nc.vector.tensor_mul(U[sl], cs_ps[sl], C12[:, h0:h1])
                nc.tensor.matmul(bm_ps[sl], ones[:], RS[sl],
                                 start=False, stop=True)
                # last chunk runs its whole tail on DVE back-to-back so the
                # Pool queue (busy with earlier sigma groups) never gates it
                eng = nc.vector if k == len(LN_CH) - 1 else nc.gpsimd
                eng.tensor_mul(V[sl], RS[sl], C2[:, h0:h1])
                eng.tensor_sub(W[sl], U[sl], V[sl])
                nc.vector.tensor_sub(bd[sl], W[sl], bm_ps[sl])
                nc.vector.tensor_scalar(bdc[sl], bd[sl], 30.0, -30.0,
                                        op0=OP.min, op1=OP.max)

            def emit_sigma(h0, h1, final=False):
                sl = (slice(None), slice(h0, h1))
                nc.scalar.activation(e1[sl], bdc[sl], AF.Exp)
                nc.vector.tensor_scalar_add(ssum[sl], e1[sl], 1.0)
                nc.vector.reciprocal(osb[:, h0:h1, 0], ssum[sl])
                nc.vector.tensor_mul(osb[:, h0:h1, 1], e1[sl],
                                     osb[:, h0:h1, 0])
                nc.sync.dma_start(o_d[:, h0:h1, :], osb[:, h0:h1, :])

            def chunk_steps(li):
                emit_ln(li)
                if li == 0:
                    emit_pre2()
                emit_chunk_post(li)

            emit_exp(0)
            emit_pre1()
            for k in range(1, len(EXP_CH)):
                emit_exp(k)
                while (li < len(LN_CH)
                       and ln_cum[li + 1] <= exp_cum[max(k + 1 - LN_DELAY,
                                                         0)]):
                    chunk_steps(li)
                    li += 1
            while li < len(LN_CH):
                chunk_steps(li)
                li += 1
            # sigma groups after the whole stream so they never stall it
            # final (critical) group first so the big group's DVE ops
            # never sit between the last bdc and the final sigma chain
            bounds = [0] + [ln_cum[li + 1] - GX for li in SIG_AT]
            if bounds[-1] < H:
                bounds.append(H)
            emit_sigma(bounds[-2], bounds[-1], final=True)
            for n in range(len(bounds) - 2):
                emit_sigma(bounds[n], bounds[n + 1])

    nc.compile()
    return nc


def _core_inputs(s_edge, s_sib, c):
    b, hs = c >> 1, (c & 1) * H
    d = np.arange(L)
    hl = np.arange(H)
    tx = np.zeros((L, EXT, L), dtype=np.float32)
    tx[:, GX:, :] = s_sib[b, :, hs:hs + H, :]
    t = tx[:, GX:, :]
    tx[:, 0, 0:H] = t[d[:, None], hl[None, :], (hs + hl)[None, :]]
    tx[:, 0, H:2 * H] = t[d[:, None], hl[None, :], d[:, None]]
    tx[:, 1, 0:H] = s_sib[
        b, (hs + hl)[None, :], (hs + hl)[None, :], d[:, None]]
    se = np.ascontiguousarray(s_edge[b, :, hs:hs + H, :], dtype=np.float32)
    aux = np.empty((L, A_COLS), dtype=np.float32)
    E = (d[:, None] == (hs + hl)[None, :]).astype(np.float32)
    aux[:, A_E:A_E + H] = E
    aux[:, A_N:A_N + H] = 126.0 + E
    aux[:, A_OME:A_OME + H] = 1.0 - E
    aux[:, A_C1:A_C1 + H] = 15878.0 + 252.0 * E
    aux[:, A_C2:A_C2 + H] = 127.0 + E
    aux[:, A_C12:A_C12 + H] = (15878.0 + 252.0 * E) + (127.0 + E)
    aux[:, A_ID:A_ID + L] = np.eye(L, dtype=np.float32)
    return {"t": tx, "se": se, "aux": aux}


def make_in_maps(s_edge, s_sib):
    return [_core_inputs(s_edge, s_sib, c) for c in range(N_CORES)]


def get_program():
    global _PROGRAM
    if _PROGRAM is None:
        _PROGRAM = _build_program()
    return _PROGRAM


def assemble(results):
    out = np.empty((4, L, L, 2), dtype=np.float32)
    for c in range(N_CORES):
        b, hs = c >> 1, (c & 1) * H
        out[b, :, hs:hs + H, :] = results[c]["o"].reshape(L, H, 2)
    return out


def kernel(s_edge, s_sib, mask):
    from concourse.bass_utils import run_bass_kernel_spmd

    s_edge = np.asarray(s_edge)
    s_sib = np.asarray(s_sib)
    mask = np.asarray(mask)
    assert mask.all(), "kernel specialized for the spec's all-ones mask"

    nc = get_program()
    in_maps = make_in_maps(s_edge, s_sib)
    res = run_bass_kernel_spmd(nc, in_maps, list(range(N_CORES))).results
    return assemble(res)
